# revision 32
# baseline (speedup 1.0000x reference)
"""Multi-head self-attention (B=2, S=2048, D=1024, H=16, causal) on 8 TRN2 cores.

Sharding: tensor-parallel over heads. Core c owns heads {2c, 2c+1}:
  - Wq/Wk/Wv column-sharded: core c gets columns [128c, 128c+128).
  - Each core computes Q^T,K^T,V^T (head-dim on partitions) for its heads,
    both batches, via bf16 matmuls against x^T; V is PE-transposed back to
    seq-on-partitions layout.  All matmul operands are bf16 (host casts the
    f32 inputs); accumulation stays f32 in PSUM.
  - Attention in transposed-scores layout: S^T[k, q] tiles, so softmax
    denominators come free from an extra ones-column in V (row 64 of the
    AV psum accumulates sum_k P^T[k, q]).  Causal masking is folded into
    the scores matmul accumulation as a -BIG upper-triangular bias matmul
    (bf16) on diagonal tiles; fully-masked column blocks are skipped.
  - Normalize Z^T by the per-q reciprocal (PE broadcast of the recip row).
  - Exchange is pipelined per batch: an AllToAll after batch-0 attention
    runs during batch-1 attention, and batch-0's quarter of the output
    projection is interleaved into the batch-1 beat schedule.  Core c owns
    256 output rows per batch (rows [256c, 256c+256) of each batch), so
    only batch-1's small AllToAll + 256-row projection sit on the tail.
"""

import ml_dtypes
import numpy as np

import concourse.bass as bass
import concourse.mybir as mybir
import concourse.tile as tile
from concourse import bacc
from concourse.bass_utils import run_bass_kernel_spmd

N_CORES = 8
B, S, D = 2, 2048, 1024
H = 16
HD = D // H          # 64
BS = B * S           # 4096 flattened tokens
CD = 2 * HD          # 128 head-dims per core
NM = S // 512        # 4 q-chunks per batch
BIG = 30000.0
SCALE = 1.0 / np.sqrt(HD)
WARM_N = 56

F32 = mybir.dt.float32
BF16 = mybir.dt.bfloat16
F32R = mybir.dt.float32r
EXP = mybir.ActivationFunctionType.Exp

_CACHE = {}


def build_nc(with_collective=True, reps=1):
    nc = bacc.Bacc("TRN2", target_bir_lowering=False, debug=False, num_devices=N_CORES)

    xT = nc.dram_tensor("xT", [D, BS], BF16, kind="ExternalInput").ap()
    wq = nc.dram_tensor("wq", [D, CD], BF16, kind="ExternalInput").ap()
    wk = nc.dram_tensor("wk", [D, CD], BF16, kind="ExternalInput").ap()
    wv = nc.dram_tensor("wv", [D, CD], BF16, kind="ExternalInput").ap()
    wo = nc.dram_tensor("wo", [D, D], BF16, kind="ExternalInput").ap()
    bo = nc.dram_tensor("bo", [1, D], BF16, kind="ExternalInput").ap()
    masku = nc.dram_tensor("masku", [128, 128], BF16, kind="ExternalInput").ap()
    ident = nc.dram_tensor("ident", [128, 128], BF16, kind="ExternalInput").ap()
    onesb = nc.dram_tensor("onesb", [128, 128], BF16, kind="ExternalInput").ap()
    onesr = nc.dram_tensor("onesr", [128, 128], F32, kind="ExternalInput").ap()
    out = nc.dram_tensor("out", [512, D], F32, kind="ExternalOutput").ap()

    with tile.TileContext(nc) as tc:
        with (
            tc.tile_pool(name="const", bufs=1) as constp,
            tc.tile_pool(name="persist", bufs=1) as persist,
            tc.tile_pool(name="xt", bufs=2) as xtp,
            tc.tile_pool(name="work", bufs=3) as work,
            tc.tile_pool(name="dram", bufs=1, space="DRAM") as dram,
        ):
            # ---- small constants (loaded once) ----
            masku_sb = constp.tile([128, 128], BF16)
            ident_sb = constp.tile([128, 128], BF16)
            onesb_sb = constp.tile([128, 128], BF16)
            ones_sb = constp.tile([128, 128], F32R)
            xTr = xT.rearrange("(e p) s -> p e s", p=128)

            for _rep in range(reps):
                _body(nc, tc, constp, persist, xtp, work, dram,
                      xTr, wq, wk, wv, wo, bo, out,
                      masku_sb, ident_sb, onesb_sb, ones_sb,
                      with_collective, (masku, ident, onesb, onesr),
                      first=(_rep == 0))

    nc.compile()
    return nc


def _body(nc, tc, constp, persist, xtp, work, dram,
          xTr, wq, wk, wv, wo, bo, out,
          masku_sb, ident_sb, onesb_sb, ones_sb,
          with_collective, const_srcs, first=True):
    # per-rep exchange staging
    cc_in = dram.tile([2, 8, 128, 256], BF16, tag="cc_in", name="cc_in")
    cc_out = dram.tile([2, 8, 128, 256], BF16, tag="cc_out", name="cc_out")
    # ---- projection weights ----
    wq_sb = constp.tile([128, 8, CD], BF16, tag="wq", name="wq_sb")
    wk_sb = constp.tile([128, 8, CD], BF16, tag="wk", name="wk_sb")
    wv_sb = constp.tile([128, 8, CD], BF16, tag="wv", name="wv_sb")
    wqr = wq.rearrange("(e p) c -> p e c", p=128)
    nc.sync.dma_start(wq_sb[:, 0:2, :], wqr[:, 0:2, :])
    nc.sync.dma_start(wq_sb[:, 2:8, :], wqr[:, 2:8, :])

    # ---- persistent activations ----
    qt_sb = persist.tile([128, BS], BF16, tag="qt", name="qt_sb")
    kt_sb = persist.tile([128, BS], BF16, tag="kt", name="kt_sb")
    v_sb = persist.tile([128, 32, 130], BF16, tag="v", name="v_sb")
    wo_sb = persist.tile([128, 8, D], BF16, tag="wo", name="wo_sb")
    bo_sb = constp.tile([1, D], BF16, tag="bo", name="bo_sb")

    P = {}  # current-phase psum pools: P["pp"], P["ps"], P["pz"]

    def v_transposes(sc, vt_t):
        for st in range(4):
            tt = 4 * sc + st
            v_ps = P["pp"].tile([128, 128], BF16, tag=P["pptag"], name=f"vtp{sc}{st}")
            nc.tensor.transpose(
                v_ps[:], vt_t[:, 128 * st:128 * st + 128], ident_sb[:],
            )
            nc.vector.tensor_copy(v_sb[:, tt, 0:64], v_ps[:, 0:64])
            nc.vector.tensor_copy(v_sb[:, tt, 65:129], v_ps[:, 64:128])

    def proj_parts(sc):
        """Yield fine-grained projection closures for one 512-token chunk."""
        sl = bass.ts(sc, 512)
        state = {}

        def load():
            xt_t = xtp.tile([128, 8, 512], BF16, tag="xt", name=f"xt{sc}")
            eng = nc.scalar if sc >= 6 else nc.sync
            eng.dma_start(xt_t[:, 0:4, :], xTr[:, 0:4, sl])
            eng.dma_start(xt_t[:, 4:8, :], xTr[:, 4:8, sl])
            state["xt"] = xt_t
            state["vt"] = xtp.tile([128, 512], BF16, tag="vtc", name=f"vtc{sc}")

        def group(w_sb, o_ap_fn, name):
            def run():
                p_ps = P["pp"].tile([128, 512], F32, tag=P["pptag"], name=f"pp{sc}{name}")
                for e in range(8):
                    nc.tensor.matmul(
                        p_ps[:], w_sb[:, e, :], state["xt"][:, e, :],
                        start=(e == 0), stop=(e == 7),
                    )
                nc.vector.tensor_copy(o_ap_fn(), p_ps[:])
            return run

        yield load
        yield group(wq_sb, lambda: qt_sb[:, sl], "q")
        yield group(wk_sb, lambda: kt_sb[:, sl], "k")
        yield group(wv_sb, lambda: state["vt"][:], "v")
        yield lambda: v_transposes(sc, state["vt"][:])

    def proj_chunk0():
        """First chunk, latency-optimized: Q/K/V psum chains interleaved
        per e-slice so the PE starts as soon as the first x/weight slices
        land, using the (still free) sA/sB slots for K/V."""
        sl = bass.ts(0, 512)
        xt_t = xtp.tile([128, 8, 512], BF16, tag="xt", name="xt0")
        vt_t = xtp.tile([128, 512], BF16, tag="vtc", name="vtc0")
        wkr = wk.rearrange("(e p) c -> p e c", p=128)
        wvr = wv.rearrange("(e p) c -> p e c", p=128)
        nc.sync.dma_start(wk_sb[:, 0:2, :], wkr[:, 0:2, :])
        nc.sync.dma_start(wv_sb[:, 0:2, :], wvr[:, 0:2, :])
        nc.sync.dma_start(xt_t[:, 0:2, :], xTr[:, 0:2, sl])
        nc.sync.dma_start(wk_sb[:, 2:8, :], wkr[:, 2:8, :])
        nc.sync.dma_start(wv_sb[:, 2:8, :], wvr[:, 2:8, :])
        for eg in range(1, 4):
            nc.sync.dma_start(
                xt_t[:, 2 * eg:2 * eg + 2, :], xTr[:, 2 * eg:2 * eg + 2, sl])
        if first:
            masku_d, ident_d, onesb_d, onesr_d = const_srcs
            nc.sync.dma_start(masku_sb[:], masku_d)
            nc.sync.dma_start(ident_sb[:], ident_d)
            nc.sync.dma_start(onesb_sb[:], onesb_d)
            nc.sync.dma_start(ones_sb[:], onesr_d.bitcast(F32R))
            nc.vector.tensor_copy(v_sb[:, :, 64], onesb_sb[:, 0:32])
            nc.vector.tensor_copy(v_sb[:, :, 129], onesb_sb[:, 0:32])
        chains = [
            (wq_sb, P["pp"].tile([128, 512], F32, tag="p", name="pq0")),
            (wk_sb, P["ps"].tile([128, 512], F32, tag="sA", name="pk0")),
            (wv_sb, P["ps"].tile([128, 512], F32, tag="sB", name="pv0")),
        ]
        for e in range(8):
            for w_sb, ps in chains:
                nc.tensor.matmul(
                    ps[:], w_sb[:, e, :], xt_t[:, e, :],
                    start=(e == 0), stop=(e == 7),
                )
        nc.vector.tensor_copy(qt_sb[:, sl], chains[0][1][:])
        nc.vector.tensor_copy(kt_sb[:, sl], chains[1][1][:])
        nc.vector.tensor_copy(vt_t[:], chains[2][1][:])
        v_transposes(0, vt_t[:])

    zdone = {}

    def attn_head_beats(stream_bmh, stream):
        """Yield one closure per beat for ONE head of one 512-token chunk.
        stream selects the psum/pt tags; the two heads of a chunk run as the
        two weave streams, so one stream's matmuls cover the other's exp
        latency.  z needs one psum bank per stream (2 total)."""
        b, m, h = stream_bmh
        q0 = 2048 * b + 512 * m
        last_t = 4 * m + 3
        state = {}
        hsl = slice(64 * h, 64 * h + 64)

        def beat(t):
            if t == 0:
                state["z"] = P["pz"].tile([65, 512], F32, tag=f"z{stream}",
                                          name=f"z{b}{m}{h}", bufs=1)
            z_ps = state["z"]

            def av(ta, pt_t):
                joa = max(0, 128 * (ta - 4 * m))
                nc.tensor.matmul(
                    z_ps[:, joa:512],
                    v_sb[:, 16 * b + ta, 65 * h:65 * h + 65],
                    pt_t[:, joa:512],
                    start=(ta == 0), stop=(ta == last_t),
                )

            k0 = 2048 * b + 128 * t
            jo = max(0, 128 * (t - 4 * m))
            pt_t = work.tile([128, 512], BF16, tag=f"pt{stream}",
                             name=f"pt{b}{m}{h}{t}", bufs=3)
            s_ps = P["ps"].tile([128, 512], F32, tag=f"s{stream}",
                                name=f"s{b}{m}{h}{t}")
            nc.tensor.matmul(
                s_ps[:, jo:512],
                kt_sb[hsl, k0:k0 + 128],
                qt_sb[hsl, q0 + jo:q0 + 512],
                start=True, stop=(t < 4 * m),
            )
            if t >= 4 * m:
                nc.tensor.matmul(
                    s_ps[:, jo:jo + 128],
                    masku_sb[:], ident_sb[:],
                    start=False, stop=True,
                )
            nc.scalar.activation(
                pt_t[:, jo:512], s_ps[:, jo:512], EXP, scale=float(SCALE),
            )
            pend = state.pop("pend", None)
            if pend is not None:
                av(*pend)
            state["pend"] = (t, pt_t)
            if t == last_t:
                av(*state.pop("pend"))
                zdone[(b, m, h)] = z_ps
                other = zdone.pop((b, m, 1 - h), None)
                if other is not None:
                    zs = [z_ps, other] if h == 0 else [other, z_ps]
                    zdone.pop((b, m, h))
                    _norm(b, m, zs, fast=(b == 1 or m == 3))

        for t in range(last_t + 1):
            yield lambda t=t: beat(t)

    def _norm(b, m, z_ps, fast=False):
        # normalize and stage for all-to-all; copy psum out (incl. denom row)
        # immediately to release the z banks, then finish from SBUF
        zcp = [work.tile([65, 512], F32, tag=f"zc{h}", name=f"zc{b}{m}{h}", bufs=2)
               for h in (0, 1)]
        for h in (0, 1):
            nc.vector.tensor_copy(zcp[h][:], z_ps[h][:])
        zt_sb = work.tile([128, 512], BF16, tag="zt", name=f"zt{b}{m}", bufs=2)
        for h in (0, 1):
            recip = work.tile([65, 512], F32R, tag="rc", name=f"rc{b}{m}{h}", bufs=2)
            with nc.allow_low_precision(reason="f32r is bitwise f32 here"):
                nc.vector.reciprocal(recip[64:65, :], zcp[h][64:65, :].bitcast(F32R))
            bc_sb = work.tile([64, 512], F32, tag="bc", name=f"bcs{b}{m}{h}", bufs=2)
            if fast:
                bc_ps = P["bc"].tile([64, 512], F32, tag=P["bctag"], name=f"bcp{b}{m}{h}", bufs=P["bcbufs"])
                nc.tensor.matmul(
                    bc_ps[:], ones_sb[64:65, 0:64], recip[64:65, :],
                    start=True, stop=True,
                )
                nc.vector.tensor_copy(bc_sb[:], bc_ps[:])
            else:
                r_dram = dram.tile([1, 512], F32, tag="rd", name=f"rd{b}{m}{h}", bufs=2)
                nc.sync.dma_start(r_dram[:], recip[64:65, :].bitcast(F32))
                nc.sync.dma_start(bc_sb[:], r_dram.broadcast_to([64, 512]))
            nc.vector.tensor_mul(
                zt_sb[64 * h:64 * h + 64, :], zcp[h][0:64, :], bc_sb[:]
            )
        # stage for all-to-all: halves of this 512-token block go to cores
        # 2m and 2m+1 (each core owns 256 rows of each batch).  Pool queue:
        # keeps the exchange path off the SP queue (which streams x loads)
        # and in program order with the collectives.
        nc.sync.dma_start(cc_in[b, 2 * m], zt_sb[:, 0:256])
        nc.sync.dma_start(cc_in[b, 2 * m + 1], zt_sb[:, 256:512])

    def exchange(b):
        if with_collective:
            nc.gpsimd.collective_compute(
                "AllToAll",
                mybir.AluOpType.bypass,
                replica_groups=[list(range(N_CORES))],
                ins=[cc_in[b].opt()],
                outs=[cc_out[b].opt()],
            )
        else:
            nc.sync.dma_start(cc_out[b], cc_in[b])

    def out_proj_parts(b):
        """Output projection for this core's 256 rows of batch b.
        Yields fillers: zt2 load, then one closure per 128-row q-tile.
        The two 512-wide column halves accumulate in the two per-stream
        "s" slots so no extra PSUM banks are needed."""
        zt2 = persist.tile([128, 8, 256], BF16, tag=f"zt2{b}", name=f"zt2_{b}")

        def load():
            nc.sync.dma_start(zt2[:], cc_out[b].rearrange("i p s -> p i s"))

        yield load
        for qt in (0, 1):
            def run(qt=qt):
                o_sb = work.tile([128, 1024], F32, tag="o", name=f"os{b}{qt}", bufs=2)
                r0 = 256 * b + 128 * qt
                for e in (0, 1):
                    esl = bass.ts(e, 512)
                    o_ps = P["ps"].tile([128, 512], F32, tag=("sA", "sB")[e],
                                        name=f"o{b}{qt}{e}")
                    nc.tensor.matmul(
                        o_ps[:], onesb_sb[0:1, 0:128], bo_sb[0:1, esl],
                        start=True, stop=False,
                    )
                    for i in range(8):
                        nc.tensor.matmul(
                            o_ps[:],
                            zt2[:, i, bass.ts(qt, 128)],
                            wo_sb[:, i, esl],
                            start=False, stop=(i == 7),
                        )
                    nc.vector.tensor_copy(o_sb[:, esl], o_ps[:])
                    nc.sync.dma_start(out[r0:r0 + 128, esl], o_sb[:, esl])
            yield run

    def weave(tasks_a, tasks_b, fillers):
        """Round-robin beats from attention streams, sprinkling filler
        closures (projection work) between rounds."""
        ia = iter(tasks_a)
        ib = iter(tasks_b)
        fi = iter(fillers)
        done_a = done_b = False
        while not (done_a and done_b):
            try:
                next(ia)()
            except StopIteration:
                done_a = True
            try:
                next(ib)()
            except StopIteration:
                done_b = True
            f = next(fi, None)
            if f is not None:
                f()
        for f in fi:
            f()

    def fillers():
        """One continuous filler schedule for the 80-round weave.
        Rounds 0-39 run batch-0 attention, 40-79 batch-1.  Pacing:
        proj chunk sc must complete before attention first reads it
        (b0 m needs sc=m by round ~4m; b1 m needs sc=4+m by round ~40+4m).
        The exchange(0) must be issued after b0m3's staging (round 39),
        and PE instructions that wait on it (out-proj b0) are delayed
        further so the in-order PE queue never stalls on the collective."""
        for sc in range(1, 6):
            yield from proj_parts(sc)          # fillers 0-24
        yield lambda: nc.sync.dma_start(
            wo_sb[:], wo.rearrange("(i p) e -> p i e", p=128))
        yield lambda: nc.sync.dma_start(bo_sb[:], bo)
        for _ in range(10):                    # 27-36
            yield lambda: None
        yield from proj_parts(6)               # 37-41
        yield lambda: exchange(0)              # 42 (Pool; b0m3 staged at ~40)
        parts = list(out_proj_parts(0))
        yield parts[0]                         # 43: zt2(0) load (Pool)
        yield from proj_parts(7)               # 44-48
        for _ in range(9):
            yield lambda: None
        yield parts[1]                         # 58: q-tile 0
        for _ in range(3):
            yield lambda: None
        yield parts[2]                         # 62: q-tile 1

    # single continuous schedule: both batches' attention as two head
    # streams, with projections / exchange / output-projection as fillers.
    # PSUM plan (8 banks): p=2, sA=2, sB=2, zA=1, zB=1.
    with (
        tc.tile_pool(name="ppP", bufs=2, space="PSUM") as ppP,
        tc.tile_pool(name="psP", bufs=2, space="PSUM") as psP,
        tc.tile_pool(name="pzP", bufs=1, space="PSUM") as pzP,
    ):
        P["pp"] = ppP
        P["pptag"] = "p"
        P["ps"] = psP
        P["pz"] = pzP
        P["bc"] = ppP
        P["bctag"] = "p"
        P["bcbufs"] = 2
        proj_chunk0()
        beats_a = [bt for bm in range(8)
                   for bt in attn_head_beats((bm // 4, bm % 4, 0), "A")]
        beats_b = [bt for bm in range(8)
                   for bt in attn_head_beats((bm // 4, bm % 4, 1), "B")]
        weave(beats_a, beats_b, list(fillers()))
        # dummy accumulation chain (never read) that keeps the PE p-state
        # high through the batch-1 exchange window, so the final output
        # projection starts at full clock instead of cold
        w_ps = P["pp"].tile([128, 512], F32, tag="p", name="warm")
        for i in range(WARM_N):
            nc.tensor.matmul(w_ps[:], onesb_sb[:], kt_sb[:, 0:512],
                             start=(i == 0), stop=(i == WARM_N - 1))
        exchange(1)
        for part in out_proj_parts(1):
            part()


def _prep_inputs(inputs, Wq, Wk, Wv, Wo, bo):
    bf = ml_dtypes.bfloat16
    x = np.asarray(inputs, dtype=np.float32).reshape(BS, D)
    xT = np.ascontiguousarray(x.T).astype(bf)
    Wq = np.asarray(Wq, dtype=np.float32).astype(bf)
    Wk = np.asarray(Wk, dtype=np.float32).astype(bf)
    Wv = np.asarray(Wv, dtype=np.float32).astype(bf)
    Wo = np.ascontiguousarray(np.asarray(Wo, dtype=np.float32)).astype(bf)
    bo = np.asarray(bo, dtype=np.float32).reshape(1, D).astype(bf)
    masku = np.triu(np.full((128, 128), -BIG, dtype=np.float32), k=1).astype(bf)
    ident = np.eye(128, dtype=np.float32).astype(bf)
    onesb = np.ones((128, 128), dtype=np.float32).astype(bf)
    onesr = np.ones((128, 128), dtype=np.float32)
    in_maps = []
    for c in range(N_CORES):
        csl = slice(CD * c, CD * (c + 1))
        in_maps.append({
            "xT": xT,
            "wq": np.ascontiguousarray(Wq[:, csl]),
            "wk": np.ascontiguousarray(Wk[:, csl]),
            "wv": np.ascontiguousarray(Wv[:, csl]),
            "wo": Wo,
            "bo": bo,
            "masku": masku,
            "ident": ident,
            "onesb": onesb,
            "onesr": onesr,
        })
    return in_maps


def kernel(inputs, Wq, Wk, Wv, Wo, bo):
    if "nc" not in _CACHE:
        _CACHE["nc"] = build_nc()
    nc = _CACHE["nc"]
    in_maps = _prep_inputs(inputs, Wq, Wk, Wv, Wo, bo)
    res = None
    for attempt in range(3):
        try:
            res = run_bass_kernel_spmd(nc, in_maps, core_ids=list(range(N_CORES)))
            break
        except Exception:
            if attempt == 2:
                raise
            import time as _time

            _time.sleep(5.0)
    # core c's out rows 0:256 are batch-0 rows [256c, 256c+256);
    # rows 256:512 are batch-1 rows [256c, 256c+256)
    full = np.empty((BS, D), dtype=np.float32)
    for c in range(N_CORES):
        o = res.results[c]["out"]
        full[256 * c:256 * c + 256] = o[0:256]
        full[S + 256 * c:S + 256 * c + 256] = o[256:512]
    return full.reshape(B, S, D)
